# revision 16
# baseline (speedup 1.0000x reference)
"""Trainium2 Bass kernel for multi-head attention (B=4, S=2048, E=768, H=12).

Sharding: 8 cores = 4 batch x 2 query-halves. Each core runs all heads for
1024 query rows of one batch element; K/V are computed for the full 2048 key
positions locally, so per-core outputs are disjoint slabs and no collective
is needed.

Device pipeline (all matmuls bf16 in / fp32 accumulate):
  1. Q^T,K^T = W_{q,k}^T-tiles (stationary) @ X^T  -> [dim, pos] layout
     V       = X^T-tiles (stationary) @ W_v        -> [pos, dim] layout
  2. scores^T[k, q] = K^T-slice.T @ Q^T  (head pairs run concurrently in the
     PE array via 64-row tile_position groups)
  3. exp eviction on ScalarE: exp(scores/8 + bias_k), bias_k = -60 where
     mask[k]==0 (key masking == per-partition bias in [k, q] layout)
  4. attnV^T[d, q] accumulated over k with lhsT = [V_h | ones]; the ones
     column makes row 64 the softmax denominator
  5. divide rows 0..63 by row 64 (reciprocal + 1-row broadcast matmul),
     cast to bf16 -> fc input [head, q] tiles
  6. out[q, :] = sum_h fcin_h.T @ W_fc[h*64:(h+1)*64, :]
"""

import sys
import numpy as np

for _p in ("/opt/trn_rl_repo", "/root/.axon_site/_ro/trn_rl_repo"):
    if _p not in sys.path:
        sys.path.append(_p)

import concourse.bass as bass
import concourse.tile as tile
from concourse import bacc
from concourse import mybir
from concourse.bass_utils import run_bass_kernel_spmd


def _ensure_ntff_hook():
    """The agent image's antenv lacks axon_hooks; shim it so trace=True can
    register the NTFF profiling hook (needed only for kernel_traced)."""
    import types
    try:
        from antenv.axon_hooks import get_axon_ntff_profile_hook  # noqa: F401
        return
    except ImportError:
        pass
    try:
        from trn_agent_boot.trn_boot import _ntff_profile_via_ctypes
        hook = _ntff_profile_via_ctypes("/opt/axon/libaxon_pjrt.so")
    except Exception:
        hook = None
    mod = types.ModuleType("antenv.axon_hooks")
    mod._hook = hook
    mod.set_axon_ntff_profile_hook = lambda h: setattr(mod, "_hook", h)
    mod.get_axon_ntff_profile_hook = lambda: mod._hook
    sys.modules["antenv.axon_hooks"] = mod


_ensure_ntff_hook()

F32 = mybir.dt.float32
BF16 = mybir.dt.bfloat16
I32 = mybir.dt.int32
ALU = mybir.AluOpType
ACTF = mybir.ActivationFunctionType

N_CORES = 8
MASK_BIAS = -60.0


def build_program(S=2048, Q=1024, H=12, D=64, debug_taps=False):
    """Emit the single-core SPMD program. Returns nc (scheduled, unlowered)."""
    E = H * D
    nE = E // 128          # contraction tiles over embed dim
    nT = S // 128          # key-position tiles
    QT = min(512, Q)       # query tile (free dim of scores matmuls)
    nJ = Q // QT
    nP = H // 2            # head pairs (two 64-dim heads share a 128 tile)
    NH = (H // 2) * D      # V projection half width (=384 for full size)

    nc = bacc.Bacc("TRN2", target_bir_lowering=False, debug=False,
                   num_devices=N_CORES)

    xT_d = nc.dram_tensor("xT", [E, S], F32, kind="ExternalInput").ap()
    xTq_d = nc.dram_tensor("xTq", [E, Q], F32, kind="ExternalInput").ap()
    mask_d = nc.dram_tensor("mask", [S], I32, kind="ExternalInput").ap()
    wq_d = nc.dram_tensor("wq", [E, E], F32, kind="ExternalInput").ap()
    wk_d = nc.dram_tensor("wk", [E, E], F32, kind="ExternalInput").ap()
    wv_d = nc.dram_tensor("wv", [E, E], F32, kind="ExternalInput").ap()
    wfc_d = nc.dram_tensor("wfc", [E, E], F32, kind="ExternalInput").ap()
    out_d = nc.dram_tensor("out", [Q, E], F32, kind="ExternalOutput").ap()

    taps = {}

    def tap(name, ap):
        if debug_taps:
            taps[name] = ap

    from contextlib import ExitStack
    with tile.TileContext(nc) as tc, ExitStack() as ctx:
        p_stage = ctx.enter_context(tc.tile_pool(name="stage", bufs=8))
        p_xt = ctx.enter_context(tc.tile_pool(name="xt", bufs=nE))
        p_xtq = ctx.enter_context(tc.tile_pool(name="xtq", bufs=nE))
        p_wq = ctx.enter_context(tc.tile_pool(name="wq", bufs=nE))
        p_wk = ctx.enter_context(tc.tile_pool(name="wk", bufs=nE))
        p_wv = ctx.enter_context(tc.tile_pool(name="wv", bufs=nE))
        p_wfc = ctx.enter_context(tc.tile_pool(name="wfc", bufs=H))
        p_v = ctx.enter_context(tc.tile_pool(name="v", bufs=nT))
        p_kt = ctx.enter_context(tc.tile_pool(name="kt", bufs=nP))
        p_qt = ctx.enter_context(tc.tile_pool(name="qt", bufs=nP))
        p_const = ctx.enter_context(tc.tile_pool(name="const", bufs=4))
        p_exp = ctx.enter_context(tc.tile_pool(name="exp", bufs=4))
        p_fcin = ctx.enter_context(tc.tile_pool(name="fcin", bufs=H + 2))
        p_recip = ctx.enter_context(tc.tile_pool(name="recip", bufs=4))
        p_avs = ctx.enter_context(tc.tile_pool(name="avs", bufs=4))
        p_osb = ctx.enter_context(tc.tile_pool(name="osb", bufs=2))
        p_acc = ctx.enter_context(tc.tile_pool(name="acc", bufs=4, space="PSUM"))
        p_sc = ctx.enter_context(tc.tile_pool(name="sc", bufs=2, space="PSUM"))

        # ---- constants: ones column + mask bias -------------------------
        ones_sb = p_const.tile([128, 64], F32, tag="const")
        nc.vector.memset(ones_sb[:, :], 1.0)
        m_i32 = p_const.tile([128, nT], I32, tag="const")
        nc.gpsimd.dma_start(m_i32[:, :], mask_d.rearrange("(t p) -> p t", p=128))
        m_f32 = p_const.tile([128, nT], F32, tag="const")
        nc.vector.tensor_copy(m_f32[:, :], m_i32[:, :])
        bias_sb = p_const.tile([128, nT], F32, tag="const")
        nc.vector.tensor_scalar(bias_sb[:, :], m_f32[:, :], 1.0, -MASK_BIAS,
                                op0=ALU.subtract, op1=ALU.mult)
        tap("bias", bias_sb[:, :])
        tap("mf32", m_f32[:, :])

        # ---- load + cast inputs to bf16 ---------------------------------
        def load_bf16(pool, dram, rows, cols, tag):
            nr = rows.stop - rows.start
            t = pool.tile([128, cols], BF16, tag=tag, name=f"{tag}_{rows.start}")
            for c0 in range(0, cols, 1024):
                cn = min(1024, cols - c0)
                st = p_stage.tile([128, 1024], F32, tag="stage", name=f"st_{tag}_{rows.start}_{c0}")
                nc.gpsimd.dma_start(st[:nr, :cn],
                                  dram[rows, c0:c0 + cn])
                nc.gpsimd.tensor_copy(t[:nr, c0:c0 + cn], st[:nr, :cn])
            return t

        wq_sb, xtq_sb, wk_sb, xt_sb, wv_sb = [], [], [], [], []
        for e in range(nE):
            wq_sb.append(load_bf16(p_wq, wq_d, slice(e * 128, (e + 1) * 128), E, "wq"))
            xtq_sb.append(load_bf16(p_xtq, xTq_d, slice(e * 128, (e + 1) * 128), Q, "xtq"))

        # ---- projections -------------------------------------------------
        def proj_T(p, dst_pool, xsrc, w_sb, ncols, tag):
            """dst[dim128, ncols] = (W tile).T @ X^T, accumulated over E."""
            dst = dst_pool.tile([128, ncols], BF16, tag=tag, name=f"{tag}{p}")
            for n0 in range(0, ncols, 512):
                nn = min(512, ncols - n0)
                ps = p_acc.tile([128, 512], F32, tag="acc", name=f"ps_{tag}{p}_{n0}")
                for e in range(nE):
                    nc.tensor.matmul(ps[:, :nn],
                                     lhsT=w_sb[e][:, p * 128:(p + 1) * 128],
                                     rhs=xsrc[e][:, n0:n0 + nn],
                                     start=(e == 0), stop=(e == nE - 1))
                nc.vector.tensor_copy(dst[:, n0:n0 + nn], ps[:, :nn])
            return dst

        def proj_v(t):
            """v tile [128 pos, H, D+1]: V in cols 0..D-1, ones in col D."""
            vt = p_v.tile([128, H, D + 1], BF16, tag="v", name=f"v{t}")
            nc.vector.memset(vt[:, :, D:D + 1], 1.0)
            for half in range(2):
                ps = p_acc.tile([128, 512], F32, tag="acc", name=f"ps_v{t}_{half}")
                for e in range(nE):
                    nc.tensor.matmul(ps[:, :NH],
                                     lhsT=xt_sb[e][:, t * 128:(t + 1) * 128],
                                     rhs=wv_sb[e][:, half * NH:(half + 1) * NH],
                                     start=(e == 0), stop=(e == nE - 1))
                nc.vector.tensor_copy(
                    vt[:, half * (H // 2):(half + 1) * (H // 2), 0:D],
                    ps[:, :NH].rearrange("p (h d) -> p h d", d=D))
            return vt

        # pair-0 Q/K first so attention (and the ScalarE exp stream) starts
        # as early as possible; remaining projections fill PE gaps later.
        qt_sb, kt_sb = {}, {}
        qt_sb[0] = proj_T(0, p_qt, xtq_sb, wq_sb, Q, "qt")
        for e in range(nE):
            wk_sb.append(load_bf16(p_wk, wk_d, slice(e * 128, (e + 1) * 128), E, "wk"))
            xt_sb.append(load_bf16(p_xt, xT_d, slice(e * 128, (e + 1) * 128), S, "xt"))
        kt_sb[0] = proj_T(0, p_kt, xt_sb, wk_sb, S, "kt")
        for e in range(nE):
            wv_sb.append(load_bf16(p_wv, wv_d, slice(e * 128, (e + 1) * 128), E, "wv"))
        wfc_sb = [load_bf16(p_wfc, wfc_d, slice(h * 64, (h + 1) * 64), E, "wfc")
                  for h in range(H)]
        v_sb = [proj_v(t) for t in range(nT)]
        for p in range(1, nP):
            qt_sb[p] = proj_T(p, p_qt, xtq_sb, wq_sb, Q, "qt")
            kt_sb[p] = proj_T(p, p_kt, xt_sb, wk_sb, S, "kt")
        tap("xt0", xt_sb[0][:, :])
        tap("wq0", wq_sb[0][:, :])
        tap("qt0", qt_sb[0][:, :])
        tap("kt0", kt_sb[0][:, :])
        tap("v0", v_sb[0][:, :, :])

        # ---- attention + fc ---------------------------------------------
        for j in range(nJ):
            jq = slice(j * QT, (j + 1) * QT)
            fcin = [p_fcin.tile([64, QT], BF16, tag="fcin", name=f"fcin{j}_{h}") for h in range(H)]
            for p in range(nP):
                av = [p_acc.tile([128, 512], F32, tag="acc", name=f"av{j}_{p}_{hi}") for hi in range(2)]
                for t in range(nT):
                    sc = p_sc.tile([128, 2 * QT], F32, tag="sc", name=f"sc{j}_{p}_{t}")
                    for hi in range(2):
                        nc.tensor.matmul(
                            sc[:, hi * QT:(hi + 1) * QT],
                            lhsT=kt_sb[p][hi * 64:(hi + 1) * 64,
                                          t * 128:(t + 1) * 128],
                            rhs=qt_sb[p][hi * 64:(hi + 1) * 64, jq],
                            start=True, stop=True)
                    ex = p_exp.tile([128, 2 * QT], BF16, tag="exp", name=f"ex{j}_{p}_{t}")
                    nc.scalar.activation(ex[:, :], sc[:, :], ACTF.Exp,
                                         bias=bias_sb[:, t:t + 1], scale=0.125)
                    if j == 0 and p == 0 and t == 0:
                        tap("ex000", ex[:, :])
                        tap("sc000", sc[:, :])
                    for hi in range(2):
                        nc.tensor.matmul(av[hi][0:D + 1, :QT],
                                         lhsT=v_sb[t][:, 2 * p + hi, :],
                                         rhs=ex[:, hi * QT:(hi + 1) * QT],
                                         start=(t == 0), stop=(t == nT - 1))
                for hi in range(2):
                    # evict attnV^T (+sums row) to SBUF, freeing the PSUM slot
                    avs = p_avs.tile([128, QT], F32, tag="avs",
                                     name=f"avs{j}_{p}_{hi}")
                    nc.vector.tensor_copy(avs[0:D + 1, :], av[hi][0:D + 1, :QT])
                    # broadcast raw sums across 64 partitions via rank-1 matmul,
                    # then 1/x = exp(-ln(x)) on ScalarE (both funcs share the
                    # natural_log_exp table set)
                    bc = p_acc.tile([128, 512], F32, tag="acc", name=f"bc{j}_{p}_{hi}")
                    nc.tensor.matmul(bc[0:64, :QT], lhsT=ones_sb[D:D + 1, 0:64],
                                     rhs=avs[D:D + 1, :], start=True, stop=True)
                    lnb = p_recip.tile([128, QT], F32, tag="recip",
                                       name=f"ln{j}_{p}_{hi}")
                    nc.scalar.activation(lnb[0:D, :], bc[0:D, :QT], ACTF.Ln)
                    rcb = p_recip.tile([128, QT], F32, tag="recip",
                                       name=f"rc{j}_{p}_{hi}")
                    nc.scalar.activation(rcb[0:D, :], lnb[0:D, :], ACTF.Exp,
                                         scale=-1.0)
                    nc.vector.tensor_tensor(fcin[2 * p + hi][:, :],
                                            avs[0:D, :QT], rcb[0:D, :],
                                            op=ALU.mult)
                    if j == 0 and p == 0 and hi == 0:
                        tap("av00", avs[:, :QT])
                        tap("rc00", rcb[:, :])
                        tap("bc00", bc[0:D, :QT])
                        tap("fcin0", fcin[0][:, :])
            for qs in range(QT // 128):
                osb = p_osb.tile([128, E], F32, tag="osb", name=f"osb{j}_{qs}")
                for half in range(2):
                    nf = E // 2
                    ps = p_acc.tile([128, 512], F32, tag="acc", name=f"ps_fc{j}_{qs}_{half}")
                    for h in range(H):
                        nc.tensor.matmul(
                            ps[:, :nf],
                            lhsT=fcin[h][:, qs * 128:(qs + 1) * 128],
                            rhs=wfc_sb[h][0:64, half * nf:(half + 1) * nf],
                            start=(h == 0), stop=(h == H - 1))
                    nc.vector.tensor_copy(osb[:, half * nf:(half + 1) * nf],
                                          ps[:, :nf])
                r0 = j * QT + qs * 128
                nc.gpsimd.dma_start(out_d[r0:r0 + 128, :], osb[:, :])
        if taps:
            with tc.tile_pool(name="dbgtap", bufs=len(taps)) as p_dbg:
                for nm, ap in taps.items():
                    td = nc.dram_tensor(f"tap_{nm}", list(ap.shape), ap.dtype,
                                        kind="ExternalOutput").ap()
                    if ap.space == bass.MemorySpace.PSUM:
                        stg = p_dbg.tile(list(ap.shape), ap.dtype, tag="dbg",
                                         name=f"dbg_{nm}")
                        nc.vector.tensor_copy(stg[...], ap)
                        ap = stg[...]
                    nc.gpsimd.dma_start(td[...], ap)
    nc.compile()
    return nc


def host_prep(X, mask, W_qkv, W_fc, Q=1024, H=12, D=64, compact=True):
    """Shard inputs for the 8 cores: core c -> batch c//2, query half c%2.

    With compact=True, each core's key set is gathered down to the unmasked
    positions (padded to a common multiple of 128); masked keys contribute
    exactly zero to the reference output, so this is a pure sharding choice.
    Returns (in_maps, S_k).
    """
    E = H * D
    B, S, _ = X.shape
    mask = np.asarray(mask)
    Wr = np.asarray(W_qkv, np.float32).reshape(E, H, 3 * D)
    wq = np.ascontiguousarray(Wr[:, :, 0:D].reshape(E, E))
    wk = np.ascontiguousarray(Wr[:, :, D:2 * D].reshape(E, E))
    wv = np.ascontiguousarray(Wr[:, :, 2 * D:3 * D].reshape(E, E))
    wfc = np.ascontiguousarray(np.asarray(W_fc, np.float32))
    if compact:
        counts = mask.sum(axis=1)
        S_k = max(128, int(-(-counts.max() // 128)) * 128)
    else:
        S_k = S
    in_maps = []
    for c in range(N_CORES):
        b, half = c // 2, c % 2
        xT = np.ascontiguousarray(np.asarray(X[b], np.float32).T)
        if compact:
            idx = np.nonzero(mask[b])[0]
            nk = len(idx)
            idxp = np.concatenate([idx, np.zeros(S_k - nk, np.int64)])
            xk = np.ascontiguousarray(xT[:, idxp])
            cm = np.zeros(S_k, np.int32)
            cm[:nk] = 1
        else:
            xk = xT
            cm = np.ascontiguousarray(mask[b].astype(np.int32))
        in_maps.append({
            "xT": xk,
            "xTq": np.ascontiguousarray(xT[:, half * Q:(half + 1) * Q]),
            "mask": cm,
            "wq": wq, "wk": wk, "wv": wv, "wfc": wfc,
        })
    return in_maps, S_k


_NC_CACHE = {}
COMPACT = True


def _get_program(S_k, Q):
    key = (S_k, Q)
    if key not in _NC_CACHE:
        _NC_CACHE[key] = build_program(S=S_k, Q=Q)
    return _NC_CACHE[key]


def _run(X, mask, W_qkv, W_fc, trace=False):
    B, S, E = X.shape
    Q = (B * S) // N_CORES
    in_maps, S_k = host_prep(X, mask, W_qkv, W_fc, Q=Q, compact=COMPACT)
    nc = _get_program(S_k, Q)
    res = run_bass_kernel_spmd(nc, in_maps, list(range(N_CORES)), trace=trace)
    out = np.empty((B, S, E), dtype=np.float32)
    for c in range(N_CORES):
        b, half = c // 2, c % 2
        out[b, half * Q:(half + 1) * Q, :] = res.results[c]["out"]
    return out, res


def kernel(X, mask, W_qkv, W_fc):
    out, _ = _run(X, mask, W_qkv, W_fc, trace=False)
    return out


def kernel_traced(X, mask, W_qkv, W_fc):
    out, res = _run(X, mask, W_qkv, W_fc, trace=True)
    return out, res


# revision 17
# speedup vs baseline: 1.1934x; 1.1934x over previous
"""Trainium2 Bass kernel for multi-head attention (B=4, S=2048, E=768, H=12).

Sharding: 8 cores = 4 batch x 2 query-halves. Each core runs all heads for
1024 query rows of one batch element; K/V are computed for the full 2048 key
positions locally, so per-core outputs are disjoint slabs and no collective
is needed.

Device pipeline (all matmuls bf16 in / fp32 accumulate):
  1. Q^T,K^T = W_{q,k}^T-tiles (stationary) @ X^T  -> [dim, pos] layout
     V       = X^T-tiles (stationary) @ W_v        -> [pos, dim] layout
  2. scores^T[k, q] = K^T-slice.T @ Q^T  (head pairs run concurrently in the
     PE array via 64-row tile_position groups)
  3. exp eviction on ScalarE: exp(scores/8 + bias_k), bias_k = -60 where
     mask[k]==0 (key masking == per-partition bias in [k, q] layout)
  4. attnV^T[d, q] accumulated over k with lhsT = [V_h | ones]; the ones
     column makes row 64 the softmax denominator
  5. divide rows 0..63 by row 64 (reciprocal + 1-row broadcast matmul),
     cast to bf16 -> fc input [head, q] tiles
  6. out[q, :] = sum_h fcin_h.T @ W_fc[h*64:(h+1)*64, :]
"""

import sys
import numpy as np

for _p in ("/opt/trn_rl_repo", "/root/.axon_site/_ro/trn_rl_repo"):
    if _p not in sys.path:
        sys.path.append(_p)

import concourse.bass as bass
import concourse.tile as tile
from concourse import bacc
from concourse import mybir
from concourse.bass_utils import run_bass_kernel_spmd


def _ensure_ntff_hook():
    """The agent image's antenv lacks axon_hooks; shim it so trace=True can
    register the NTFF profiling hook (needed only for kernel_traced)."""
    import types
    try:
        from antenv.axon_hooks import get_axon_ntff_profile_hook  # noqa: F401
        return
    except ImportError:
        pass
    try:
        from trn_agent_boot.trn_boot import _ntff_profile_via_ctypes
        hook = _ntff_profile_via_ctypes("/opt/axon/libaxon_pjrt.so")
    except Exception:
        hook = None
    mod = types.ModuleType("antenv.axon_hooks")
    mod._hook = hook
    mod.set_axon_ntff_profile_hook = lambda h: setattr(mod, "_hook", h)
    mod.get_axon_ntff_profile_hook = lambda: mod._hook
    sys.modules["antenv.axon_hooks"] = mod


_ensure_ntff_hook()

F32 = mybir.dt.float32
BF16 = mybir.dt.bfloat16
I32 = mybir.dt.int32
ALU = mybir.AluOpType
ACTF = mybir.ActivationFunctionType

N_CORES = 8
MASK_BIAS = -60.0


def build_program(S=2048, Q=1024, H=12, D=64, debug_taps=False):
    """Emit the single-core SPMD program. Returns nc (scheduled, unlowered)."""
    E = H * D
    nE = E // 128          # contraction tiles over embed dim
    nT = S // 128          # key-position tiles
    QT = min(512, Q)       # query tile (free dim of scores matmuls)
    nJ = Q // QT
    nP = H // 2            # head pairs (two 64-dim heads share a 128 tile)
    NH = (H // 2) * D      # V projection half width (=384 for full size)

    nc = bacc.Bacc("TRN2", target_bir_lowering=False, debug=False,
                   num_devices=N_CORES)

    xT_d = nc.dram_tensor("xT", [E, S], F32, kind="ExternalInput").ap()
    xTq_d = nc.dram_tensor("xTq", [E, Q], F32, kind="ExternalInput").ap()
    mask_d = nc.dram_tensor("mask", [S], I32, kind="ExternalInput").ap()
    wq_d = nc.dram_tensor("wq", [E, E], F32, kind="ExternalInput").ap()
    wk_d = nc.dram_tensor("wk", [E, E], F32, kind="ExternalInput").ap()
    wv_d = nc.dram_tensor("wv", [E, E], F32, kind="ExternalInput").ap()
    wfc_d = nc.dram_tensor("wfc", [E, E], F32, kind="ExternalInput").ap()
    out_d = nc.dram_tensor("out", [Q, E], F32, kind="ExternalOutput").ap()

    taps = {}

    def tap(name, ap):
        if debug_taps:
            taps[name] = ap

    from contextlib import ExitStack
    with tile.TileContext(nc) as tc, ExitStack() as ctx:
        p_stage = ctx.enter_context(tc.tile_pool(name="stage", bufs=8))
        p_xt = ctx.enter_context(tc.tile_pool(name="xt", bufs=nE))
        p_xtq = ctx.enter_context(tc.tile_pool(name="xtq", bufs=nE))
        p_wq = ctx.enter_context(tc.tile_pool(name="wq", bufs=nE))
        p_wk = ctx.enter_context(tc.tile_pool(name="wk", bufs=nE))
        p_wv = ctx.enter_context(tc.tile_pool(name="wv", bufs=nE))
        p_wfc = ctx.enter_context(tc.tile_pool(name="wfc", bufs=H))
        p_v = ctx.enter_context(tc.tile_pool(name="v", bufs=nT))
        p_kt = ctx.enter_context(tc.tile_pool(name="kt", bufs=nP))
        p_qt = ctx.enter_context(tc.tile_pool(name="qt", bufs=nP))
        p_const = ctx.enter_context(tc.tile_pool(name="const", bufs=4))
        p_exp = ctx.enter_context(tc.tile_pool(name="exp", bufs=4))
        p_fcin = ctx.enter_context(tc.tile_pool(name="fcin", bufs=H + 2))
        p_recip = ctx.enter_context(tc.tile_pool(name="recip", bufs=4))
        p_avs = ctx.enter_context(tc.tile_pool(name="avs", bufs=4))
        p_osb = ctx.enter_context(tc.tile_pool(name="osb", bufs=2))
        p_acc = ctx.enter_context(tc.tile_pool(name="acc", bufs=4, space="PSUM"))
        p_sc = ctx.enter_context(tc.tile_pool(name="sc", bufs=2, space="PSUM"))

        # ---- constants: ones column + mask bias -------------------------
        ones_sb = p_const.tile([128, 64], F32, tag="const")
        nc.vector.memset(ones_sb[:, :], 1.0)
        m_i32 = p_const.tile([128, nT], I32, tag="const")
        nc.gpsimd.dma_start(m_i32[:, :], mask_d.rearrange("(t p) -> p t", p=128))
        m_f32 = p_const.tile([128, nT], F32, tag="const")
        nc.vector.tensor_copy(m_f32[:, :], m_i32[:, :])
        bias_sb = p_const.tile([128, nT], F32, tag="const")
        nc.vector.tensor_scalar(bias_sb[:, :], m_f32[:, :], 1.0, -MASK_BIAS,
                                op0=ALU.subtract, op1=ALU.mult)
        tap("bias", bias_sb[:, :])
        tap("mf32", m_f32[:, :])

        # ---- load + cast inputs to bf16 ---------------------------------
        def load_bf16(pool, dram, rows, cols, tag):
            nr = rows.stop - rows.start
            t = pool.tile([128, cols], BF16, tag=tag, name=f"{tag}_{rows.start}")
            for c0 in range(0, cols, 1024):
                cn = min(1024, cols - c0)
                st = p_stage.tile([128, 1024], F32, tag="stage", name=f"st_{tag}_{rows.start}_{c0}")
                nc.gpsimd.dma_start(st[:nr, :cn],
                                  dram[rows, c0:c0 + cn])
                nc.vector.tensor_copy(t[:nr, c0:c0 + cn], st[:nr, :cn])
            return t

        wq_sb, xtq_sb, wk_sb, xt_sb, wv_sb = [], [], [], [], []
        for e in range(nE):
            wq_sb.append(load_bf16(p_wq, wq_d, slice(e * 128, (e + 1) * 128), E, "wq"))
            xtq_sb.append(load_bf16(p_xtq, xTq_d, slice(e * 128, (e + 1) * 128), Q, "xtq"))

        # ---- projections -------------------------------------------------
        def proj_T(p, dst_pool, xsrc, w_sb, ncols, tag):
            """dst[dim128, ncols] = (W tile).T @ X^T, accumulated over E."""
            dst = dst_pool.tile([128, ncols], BF16, tag=tag, name=f"{tag}{p}")
            for n0 in range(0, ncols, 512):
                nn = min(512, ncols - n0)
                ps = p_acc.tile([128, 512], F32, tag="acc", name=f"ps_{tag}{p}_{n0}")
                for e in range(nE):
                    nc.tensor.matmul(ps[:, :nn],
                                     lhsT=w_sb[e][:, p * 128:(p + 1) * 128],
                                     rhs=xsrc[e][:, n0:n0 + nn],
                                     start=(e == 0), stop=(e == nE - 1))
                nc.vector.tensor_copy(dst[:, n0:n0 + nn], ps[:, :nn])
            return dst

        def proj_v(t):
            """v tile [128 pos, H, D+1]: V in cols 0..D-1, ones in col D."""
            vt = p_v.tile([128, H, D + 1], BF16, tag="v", name=f"v{t}")
            nc.vector.memset(vt[:, :, D:D + 1], 1.0)
            for half in range(2):
                ps = p_acc.tile([128, 512], F32, tag="acc", name=f"ps_v{t}_{half}")
                for e in range(nE):
                    nc.tensor.matmul(ps[:, :NH],
                                     lhsT=xt_sb[e][:, t * 128:(t + 1) * 128],
                                     rhs=wv_sb[e][:, half * NH:(half + 1) * NH],
                                     start=(e == 0), stop=(e == nE - 1))
                nc.vector.tensor_copy(
                    vt[:, half * (H // 2):(half + 1) * (H // 2), 0:D],
                    ps[:, :NH].rearrange("p (h d) -> p h d", d=D))
            return vt

        # pair-0 Q/K first so attention (and the ScalarE exp stream) starts
        # as early as possible; remaining projections fill PE gaps later.
        qt_sb, kt_sb = {}, {}
        qt_sb[0] = proj_T(0, p_qt, xtq_sb, wq_sb, Q, "qt")
        for e in range(nE):
            wk_sb.append(load_bf16(p_wk, wk_d, slice(e * 128, (e + 1) * 128), E, "wk"))
            xt_sb.append(load_bf16(p_xt, xT_d, slice(e * 128, (e + 1) * 128), S, "xt"))
        kt_sb[0] = proj_T(0, p_kt, xt_sb, wk_sb, S, "kt")
        for e in range(nE):
            wv_sb.append(load_bf16(p_wv, wv_d, slice(e * 128, (e + 1) * 128), E, "wv"))
        wfc_sb = [load_bf16(p_wfc, wfc_d, slice(h * 64, (h + 1) * 64), E, "wfc")
                  for h in range(H)]
        v_sb = [proj_v(t) for t in range(nT)]
        for p in range(1, nP):
            qt_sb[p] = proj_T(p, p_qt, xtq_sb, wq_sb, Q, "qt")
            kt_sb[p] = proj_T(p, p_kt, xt_sb, wk_sb, S, "kt")
        tap("xt0", xt_sb[0][:, :])
        tap("wq0", wq_sb[0][:, :])
        tap("qt0", qt_sb[0][:, :])
        tap("kt0", kt_sb[0][:, :])
        tap("v0", v_sb[0][:, :, :])

        # ---- attention + fc ---------------------------------------------
        for j in range(nJ):
            jq = slice(j * QT, (j + 1) * QT)
            fcin = [p_fcin.tile([64, QT], BF16, tag="fcin", name=f"fcin{j}_{h}") for h in range(H)]
            for p in range(nP):
                av = [p_acc.tile([128, 512], F32, tag="acc", name=f"av{j}_{p}_{hi}") for hi in range(2)]
                for t in range(nT):
                    sc = p_sc.tile([128, 2 * QT], F32, tag="sc", name=f"sc{j}_{p}_{t}")
                    for hi in range(2):
                        nc.tensor.matmul(
                            sc[:, hi * QT:(hi + 1) * QT],
                            lhsT=kt_sb[p][hi * 64:(hi + 1) * 64,
                                          t * 128:(t + 1) * 128],
                            rhs=qt_sb[p][hi * 64:(hi + 1) * 64, jq],
                            start=True, stop=True)
                    ex = p_exp.tile([128, 2 * QT], BF16, tag="exp", name=f"ex{j}_{p}_{t}")
                    nc.scalar.activation(ex[:, :], sc[:, :], ACTF.Exp,
                                         bias=bias_sb[:, t:t + 1], scale=0.125)
                    if j == 0 and p == 0 and t == 0:
                        tap("ex000", ex[:, :])
                        tap("sc000", sc[:, :])
                    for hi in range(2):
                        nc.tensor.matmul(av[hi][0:D + 1, :QT],
                                         lhsT=v_sb[t][:, 2 * p + hi, :],
                                         rhs=ex[:, hi * QT:(hi + 1) * QT],
                                         start=(t == 0), stop=(t == nT - 1))
                for hi in range(2):
                    # evict attnV^T (+sums row) to SBUF, freeing the PSUM slot
                    avs = p_avs.tile([128, QT], F32, tag="avs",
                                     name=f"avs{j}_{p}_{hi}")
                    nc.vector.tensor_copy(avs[0:D + 1, :], av[hi][0:D + 1, :QT])
                    # 1/sums: spread the QT sums across partitions (DVE
                    # reciprocal is free-dim serial), invert, bring them back
                    sp = p_recip.tile([128, QT // 128], F32, tag="sp",
                                      name=f"sp{j}_{p}_{hi}")
                    nc.gpsimd.dma_start(sp[:, :], avs[D:D + 1, :])
                    rp = p_recip.tile([128, QT // 128], F32, tag="sp",
                                      name=f"rp{j}_{p}_{hi}")
                    nc.vector.reciprocal(rp[:, :], sp[:, :])
                    rr = p_recip.tile([128, QT], F32, tag="recip",
                                      name=f"rr{j}_{p}_{hi}")
                    nc.gpsimd.dma_start(rr[0:1, :], rp[:, :])
                    # broadcast 1/sums across 64 partitions via rank-1 matmul
                    bc = p_acc.tile([128, 512], F32, tag="acc", name=f"bc{j}_{p}_{hi}")
                    nc.tensor.matmul(bc[0:64, :QT], lhsT=ones_sb[0:1, 0:64],
                                     rhs=rr[0:1, :], start=True, stop=True)
                    nc.vector.tensor_tensor(fcin[2 * p + hi][:, :],
                                            avs[0:D, :QT], bc[0:D, :QT],
                                            op=ALU.mult)
                    if j == 0 and p == 0 and hi == 0:
                        tap("av00", avs[:, :QT])
                        tap("rc00", rr[:, :])
                        tap("bc00", bc[0:D, :QT])
                        tap("fcin0", fcin[0][:, :])
            for qs in range(QT // 128):
                osb = p_osb.tile([128, E], F32, tag="osb", name=f"osb{j}_{qs}")
                for half in range(2):
                    nf = E // 2
                    ps = p_acc.tile([128, 512], F32, tag="acc", name=f"ps_fc{j}_{qs}_{half}")
                    for h in range(H):
                        nc.tensor.matmul(
                            ps[:, :nf],
                            lhsT=fcin[h][:, qs * 128:(qs + 1) * 128],
                            rhs=wfc_sb[h][0:64, half * nf:(half + 1) * nf],
                            start=(h == 0), stop=(h == H - 1))
                    nc.vector.tensor_copy(osb[:, half * nf:(half + 1) * nf],
                                          ps[:, :nf])
                r0 = j * QT + qs * 128
                nc.gpsimd.dma_start(out_d[r0:r0 + 128, :], osb[:, :])
        if taps:
            with tc.tile_pool(name="dbgtap", bufs=len(taps)) as p_dbg:
                for nm, ap in taps.items():
                    td = nc.dram_tensor(f"tap_{nm}", list(ap.shape), ap.dtype,
                                        kind="ExternalOutput").ap()
                    if ap.space == bass.MemorySpace.PSUM:
                        stg = p_dbg.tile(list(ap.shape), ap.dtype, tag="dbg",
                                         name=f"dbg_{nm}")
                        nc.vector.tensor_copy(stg[...], ap)
                        ap = stg[...]
                    nc.gpsimd.dma_start(td[...], ap)
    nc.compile()
    return nc


def host_prep(X, mask, W_qkv, W_fc, Q=1024, H=12, D=64, compact=True):
    """Shard inputs for the 8 cores: core c -> batch c//2, query half c%2.

    With compact=True, each core's key set is gathered down to the unmasked
    positions (padded to a common multiple of 128); masked keys contribute
    exactly zero to the reference output, so this is a pure sharding choice.
    Returns (in_maps, S_k).
    """
    E = H * D
    B, S, _ = X.shape
    mask = np.asarray(mask)
    Wr = np.asarray(W_qkv, np.float32).reshape(E, H, 3 * D)
    wq = np.ascontiguousarray(Wr[:, :, 0:D].reshape(E, E))
    wk = np.ascontiguousarray(Wr[:, :, D:2 * D].reshape(E, E))
    wv = np.ascontiguousarray(Wr[:, :, 2 * D:3 * D].reshape(E, E))
    wfc = np.ascontiguousarray(np.asarray(W_fc, np.float32))
    if compact:
        counts = mask.sum(axis=1)
        S_k = max(128, int(-(-counts.max() // 128)) * 128)
    else:
        S_k = S
    in_maps = []
    for c in range(N_CORES):
        b, half = c // 2, c % 2
        xT = np.ascontiguousarray(np.asarray(X[b], np.float32).T)
        if compact:
            idx = np.nonzero(mask[b])[0]
            nk = len(idx)
            idxp = np.concatenate([idx, np.zeros(S_k - nk, np.int64)])
            xk = np.ascontiguousarray(xT[:, idxp])
            cm = np.zeros(S_k, np.int32)
            cm[:nk] = 1
        else:
            xk = xT
            cm = np.ascontiguousarray(mask[b].astype(np.int32))
        in_maps.append({
            "xT": xk,
            "xTq": np.ascontiguousarray(xT[:, half * Q:(half + 1) * Q]),
            "mask": cm,
            "wq": wq, "wk": wk, "wv": wv, "wfc": wfc,
        })
    return in_maps, S_k


_NC_CACHE = {}
COMPACT = True


def _get_program(S_k, Q):
    key = (S_k, Q)
    if key not in _NC_CACHE:
        _NC_CACHE[key] = build_program(S=S_k, Q=Q)
    return _NC_CACHE[key]


def _run(X, mask, W_qkv, W_fc, trace=False):
    B, S, E = X.shape
    Q = (B * S) // N_CORES
    in_maps, S_k = host_prep(X, mask, W_qkv, W_fc, Q=Q, compact=COMPACT)
    nc = _get_program(S_k, Q)
    res = run_bass_kernel_spmd(nc, in_maps, list(range(N_CORES)), trace=trace)
    out = np.empty((B, S, E), dtype=np.float32)
    for c in range(N_CORES):
        b, half = c // 2, c % 2
        out[b, half * Q:(half + 1) * Q, :] = res.results[c]["out"]
    return out, res


def kernel(X, mask, W_qkv, W_fc):
    out, _ = _run(X, mask, W_qkv, W_fc, trace=False)
    return out


def kernel_traced(X, mask, W_qkv, W_fc):
    out, res = _run(X, mask, W_qkv, W_fc, trace=True)
    return out, res
